# revision 9
# baseline (speedup 1.0000x reference)
"""3-layer GCN (PyG GCNConv semantics) on 8 Trainium2 NeuronCores.

Contract: kernel(**inputs) takes the FULL inputs (x [50000,128] f32,
edge_index [2,800000] int, W1/b1/W2/b2/W3/b3) and returns the FULL
output [50000, 64] f32.

Design: nodes are partitioned across the 8 cores by destination (6250
rows each).  Per layer every core keeps the FULL node-feature table
SBUF-resident (50000 x 128 fp16 ~ 12.8MB) in a token layout and expands
per-edge messages with SBUF-source transposed dma_gather (~1ns/desc vs
~4.7ns/desc for random HBM reads).  Norms are folded into the table
rows (t_u = dinv_u * h_u) so per-edge weights vanish: for edge (s,d)
msg = dinv_d * t_s, and self-loops are ordinary (u,u) edges.  Per edge
tile the fp16 PE computes P[e,fo] = M^T[f,e]^T W[f,fo] (fused
transpose+transform of the gathered strip) and agg[d,fo] += S01[e,d]^T
P[e,fo] with a batched 0/1 one-hot S built by a single broadcast DVE
is_equal per chunk.  Bias enters as a rank-1 matmul (deg[d] (x) b[fo])
into the same PSUM bank; the block epilogue is one scalar-engine
activation Relu(scale=dinv^2) producing the next layer's table rows.
Between layers the 0.8MB per-core shard is AllGathered and the table
re-loaded into SBUF with fully contiguous 6.4KB strides; two A-region
table buffers alternate so the A reload overlaps the B half's compute.
The table is split in two regions (A: local rows [0,3200) of each
core, B: the rest padded to 3072) to keep gather indices in int16.
"""

import numpy as np

FEAT = 128
N_CORES = 8
ABLK = 17            # blocks per core in table region A
CHUNK_BLOCKS = 1


# ---------------------------------------------------------------- host side

def preprocess(edge_index: np.ndarray, n_nodes: int, n_cores: int = N_CORES,
               chunk_blocks: int = CHUNK_BLOCKS):
    src = np.asarray(edge_index[0], dtype=np.int64)
    dst = np.asarray(edge_index[1], dtype=np.int64)
    deg = (np.bincount(dst, minlength=n_nodes) + 1).astype(np.float64)
    dinv = 1.0 / np.sqrt(deg)

    loops = np.arange(n_nodes, dtype=np.int64)
    s = np.concatenate([src, loops])
    d = np.concatenate([dst, loops])

    S_pc = n_nodes // n_cores
    assert S_pc * n_cores == n_nodes
    nblocks = (S_pc + 127) // 128
    RA = ABLK * 128                       # 3200 local rows in region A
    rka = ABLK                            # ranks per core, region A
    rkb = nblocks - ABLK                  # 24 -> 3072 padded local rows in B
    RB = S_pc - RA                        # 3050 real local rows in B
    assert n_cores * rka * 128 <= 32768 and n_cores * rkb * 128 <= 32768

    cs, ls = s // S_pc, s % S_pc
    in_a = ls < RA
    lb = ls - RA
    idx = np.where(in_a,
                   (cs * rka + (ls >> 7)) * 128 + (ls & 127),
                   (cs * rkb + (lb >> 7)) * 128 + (lb & 127))
    half = (~in_a).astype(np.int64)
    core = d // S_pc
    dl = d - core * S_pc
    blk = dl >> 7
    dloc = (dl & 127).astype(np.int64)

    counts = np.zeros((n_cores, nblocks, 2), dtype=np.int64)
    np.add.at(counts, (core, blk, half), 1)
    T = -(-counts.max(axis=0) // 128)     # [nblocks, 2] tiles per block/half
    T_lo, T_hi = T[:, 0].astype(int), T[:, 1].astype(int)
    NT_lo, NT_hi = int(T_lo.sum()), int(T_hi.sum())
    cum_lo = np.concatenate([[0], np.cumsum(T_lo)]).astype(int)
    cum_hi = np.concatenate([[0], np.cumsum(T_hi)]).astype(int)

    order = np.lexsort((idx, half, blk, core))
    idx_o, dl_o = idx[order], dloc[order]
    key = (core[order] * nblocks + blk[order]) * 2 + half[order]
    bounds = np.searchsorted(key, np.arange(n_cores * nblocks * 2 + 1))

    def wrap(a):  # [n] int -> [128, n//16] i16 wrap layout
        n = len(a)
        if n == 0:
            return np.zeros((128, 1), dtype=np.int16)
        w = a.reshape(n // 16, 16).T
        return np.tile(w, (8, 1)).copy()

    per_core = []
    for r in range(n_cores):
        idx_lo = np.zeros(128 * max(NT_lo, 1), dtype=np.int16)
        idx_hi = np.zeros(128 * max(NT_hi, 1), dtype=np.int16)
        dloc_lo = np.full((128, max(NT_lo, 1)), -1.0, dtype=np.float16)
        dloc_hi = np.full((128, max(NT_hi, 1)), -1.0, dtype=np.float16)
        for b in range(nblocks):
            for h in range(2):
                k = (r * nblocks + b) * 2 + h
                lo_, hi_ = bounds[k], bounds[k + 1]
                cnt = hi_ - lo_
                t0 = cum_lo[b] if h == 0 else cum_hi[b]
                iarr = idx_lo if h == 0 else idx_hi
                darr = dloc_lo if h == 0 else dloc_hi
                iarr[128 * t0: 128 * t0 + cnt] = idx_o[lo_:hi_].astype(np.int16)
                e = np.arange(cnt)
                darr[e % 128, t0 + e // 128] = dl_o[lo_:hi_]

        # per-block per-partition scales (pad partitions of last block -> 0)
        p_all = np.arange(nblocks * 128)
        ok = p_all < S_pc
        dv = np.zeros(nblocks * 128, dtype=np.float64)
        dv[ok] = dinv[r * S_pc + p_all[ok]]
        dinv_blk = np.ascontiguousarray(
            dv.reshape(nblocks, 128).T.astype(np.float32))
        dinv2_blk = np.ascontiguousarray(
            (dv ** 2).reshape(nblocks, 128).T.astype(np.float32))
        # rank-1 bias rows: row0 = deg (layers 0,1: 1/dinv^2), row1 = sqrt(deg)
        rdeg = np.zeros((128, nblocks * 128), dtype=np.float16)
        dgv = np.zeros(nblocks * 128, dtype=np.float64)
        dgv[ok] = deg[r * S_pc + p_all[ok]]
        rdeg[0, :] = dgv.astype(np.float16)
        rdeg[32, :] = np.sqrt(dgv).astype(np.float16)
        per_core.append(dict(
            idx_lo=wrap(idx_lo), idx_hi=wrap(idx_hi),
            dloc_lo=dloc_lo, dloc_hi=dloc_hi,
            dinv_blk=dinv_blk, dinv2_blk=dinv2_blk, rdeg=rdeg,
        ))

    # table build maps: flat A row (c,p,r) -> node id; B likewise with pad mask
    c = np.arange(n_cores)[:, None, None]
    p = np.arange(128)[None, :, None]
    ra = np.arange(rka)[None, None, :]
    amap = (c * S_pc + ra * 128 + p).reshape(-1)
    rb = np.arange(rkb)[None, None, :]
    brow = RA + rb * 128 + p
    bmask = (brow < S_pc)
    bmap = (c * S_pc + np.minimum(brow, S_pc - 1)).reshape(-1)
    bmask = np.broadcast_to(bmask, (n_cores, 128, rkb)).reshape(-1)

    chunks = [list(range(cc, min(cc + chunk_blocks, nblocks)))
              for cc in range(0, nblocks, chunk_blocks)]
    sched = dict(
        n_nodes=n_nodes, n_cores=n_cores, S_pc=S_pc, nblocks=nblocks,
        rka=rka, rkb=rkb, RA=RA, RB=RB,
        T_lo=T_lo, T_hi=T_hi, cum_lo=cum_lo, cum_hi=cum_hi,
        n_tiles_lo=NT_lo, n_tiles_hi=NT_hi, chunks=chunks,
        dinv=dinv.astype(np.float32), amap=amap, bmap=bmap, bmask=bmask,
    )
    return sched, per_core


def make_inputs(sched, per_core, x, Ws, bs):
    n_cores = sched["n_cores"]
    dinv = sched["dinv"]
    t0 = (np.asarray(x, np.float32) * dinv[:, None]).astype(np.float16)
    xa = np.ascontiguousarray(t0[sched["amap"]])
    xb = t0[sched["bmap"]].copy()
    xb[~sched["bmask"]] = 0
    iota = np.tile(np.arange(128, dtype=np.float16)[None, :], (128, 1))
    in_maps = []
    for r in range(n_cores):
        m = dict(
            xa=xa, xb=xb, iota=iota,
            idx_lo=per_core[r]["idx_lo"], idx_hi=per_core[r]["idx_hi"],
            dloc_lo=per_core[r]["dloc_lo"], dloc_hi=per_core[r]["dloc_hi"],
            dinv_blk=per_core[r]["dinv_blk"],
            dinv2_blk=per_core[r]["dinv2_blk"],
            rdeg=per_core[r]["rdeg"],
        )
        for i, (W, b) in enumerate(zip(Ws, bs)):
            m[f"W{i}"] = np.asarray(W).astype(np.float16)
            m[f"b{i}"] = np.asarray(b, dtype=np.float16)[None, :]
        in_maps.append(m)
    return in_maps


# ---------------------------------------------------------------- device side

def build_nc(sched, fos=(128, 128, 64), n_cores=None, model=False,
             compile=True, reps=1, swdge_queues=1, probe=None):
    import concourse.bacc as bacc
    import concourse.tile as tile
    import concourse.mybir as mybir

    f16, f32, i16 = mybir.dt.float16, mybir.dt.float32, mybir.dt.int16
    N, S_pc = sched["n_nodes"], sched["S_pc"]
    nblocks, rka, rkb = sched["nblocks"], sched["rka"], sched["rkb"]
    T_lo, T_hi = sched["T_lo"], sched["T_hi"]
    cum_lo, cum_hi = sched["cum_lo"], sched["cum_hi"]
    NT_lo, NT_hi = sched["n_tiles_lo"], sched["n_tiles_hi"]
    chunks = sched["chunks"]
    n_cores = n_cores or sched["n_cores"]
    n_layers = len(fos)
    NRA, NRB = n_cores * rka * 128, n_cores * rkb * 128   # table rows

    nc = bacc.Bacc("TRN2", target_bir_lowering=False, debug=False,
                   num_devices=n_cores, num_swdge_queues=swdge_queues)

    xa_in = nc.dram_tensor("xa", [NRA, FEAT], f16, kind="ExternalInput")
    xb_in = nc.dram_tensor("xb", [NRB, FEAT], f16, kind="ExternalInput")
    iota_in = nc.dram_tensor("iota", [128, 128], f16, kind="ExternalInput")
    idx_lo_in = nc.dram_tensor("idx_lo", [128, max(NT_lo, 1) * 8], i16, kind="ExternalInput")
    idx_hi_in = nc.dram_tensor("idx_hi", [128, max(NT_hi, 1) * 8], i16, kind="ExternalInput")
    dloc_lo_in = nc.dram_tensor("dloc_lo", [128, max(NT_lo, 1)], f16, kind="ExternalInput")
    dloc_hi_in = nc.dram_tensor("dloc_hi", [128, max(NT_hi, 1)], f16, kind="ExternalInput")
    dinv_in = nc.dram_tensor("dinv_blk", [128, nblocks], f32, kind="ExternalInput")
    dinv2_in = nc.dram_tensor("dinv2_blk", [128, nblocks], f32, kind="ExternalInput")
    rdeg_in = nc.dram_tensor("rdeg", [128, nblocks * 128], f16, kind="ExternalInput")
    W_in = [nc.dram_tensor(f"W{i}", [FEAT, fos[i]], f16, kind="ExternalInput")
            for i in range(n_layers)]
    b_in = [nc.dram_tensor(f"b{i}", [1, fos[i]], f16, kind="ExternalInput")
            for i in range(n_layers)]
    y_out = nc.dram_tensor("y", [S_pc, fos[-1]], f32, kind="ExternalOutput")

    rg = [list(range(n_cores))]

    with tile.TileContext(nc) as tc:
        with (
            tc.tile_pool(name="const", bufs=1) as cpool,
            tc.tile_pool(name="sb", bufs=4) as sb,
            tc.tile_pool(name="ps", bufs=24) as pspool,
            tc.tile_pool(name="mbuf", bufs=2) as mbuf,
            tc.tile_pool(name="spool", bufs=2) as spool,
            tc.tile_pool(name="psum_p", bufs=6, space="PSUM") as psum_p,
            tc.tile_pool(name="psum_agg", bufs=2, space="PSUM") as psum_agg,
            tc.tile_pool(name="dram", bufs=2, space="DRAM") as dram,
            tc.tile_pool(name="dram_sh", bufs=2, space="DRAM") as dram_sh,
        ):
            # --- constants
            iota = cpool.tile([128, 128], f16)
            nc.sync.dma_start(out=iota[:], in_=iota_in[:])
            idx_lo = cpool.tile([128, max(NT_lo, 1) * 8], i16)
            nc.sync.dma_start(out=idx_lo[:], in_=idx_lo_in[:])
            idx_hi = cpool.tile([128, max(NT_hi, 1) * 8], i16)
            nc.sync.dma_start(out=idx_hi[:], in_=idx_hi_in[:])
            dloc_lo = cpool.tile([128, max(NT_lo, 1)], f16)
            nc.sync.dma_start(out=dloc_lo[:], in_=dloc_lo_in[:])
            dloc_hi = cpool.tile([128, max(NT_hi, 1)], f16)
            nc.sync.dma_start(out=dloc_hi[:], in_=dloc_hi_in[:])
            dinv_blk = cpool.tile([128, nblocks], f32)
            nc.sync.dma_start(out=dinv_blk[:], in_=dinv_in[:])
            dinv2_blk = cpool.tile([128, nblocks], f32)
            nc.sync.dma_start(out=dinv2_blk[:], in_=dinv2_in[:])
            rdeg = cpool.tile([128, nblocks * 128], f16)
            nc.sync.dma_start(out=rdeg[:], in_=rdeg_in[:])
            Wt, bt = [], []
            for i in range(n_layers):
                w = cpool.tile([FEAT, fos[i]], f16, tag=f"W{i}")
                nc.sync.dma_start(out=w[:], in_=W_in[i][:])
                Wt.append(w)
                b = cpool.tile([128, fos[i]], f16, tag=f"b{i}")
                nc.vector.memset(b[:], 0.0)
                nc.sync.dma_start(out=b[0:1, :], in_=b_in[i][:])
                nc.sync.dma_start(out=b[32:33, :], in_=b_in[i][:])
                bt.append(b)

            # --- SBUF tables: two A buffers (alternate per layer) + one B
            tblA = []
            for i in range(2):
                tA = cpool.tile([128, n_cores * rka * FEAT], f16,
                                tag=f"tblA{i}", name=f"tblA{i}")
                tblA.append(tA)
            tblB = cpool.tile([128, n_cores * rkb * FEAT], f16, tag="tblB")

            def load_tbl(dst, src, rk):
                # src flat (c,p,r) rows of 256B -> dst[p, (c r)*256B]
                nc.sync.dma_start(
                    out=dst[:].rearrange("p (c r f) -> p c r f",
                                         c=n_cores, r=rk),
                    in_=src[:].rearrange("(c p r) f -> p c r f",
                                         p=128, r=rk))

            gq = [0]

            def emit_gather(M, tbl, idx_sb, t0, nt):
                gq[0] = (gq[0] + 1) % swdge_queues
                nc.gpsimd.dma_gather(
                    out_ap=M[:].rearrange("p (o e) -> p o e", o=1),
                    in_ap=tbl[:],
                    idxs_ap=idx_sb[:, t0 * 8:(t0 + nt) * 8],
                    num_idxs=128 * nt, num_idxs_reg=128 * nt,
                    elem_size=FEAT, transpose=True,
                    sbuf_tokens_per_rank=128,
                    sbuf_free_dim_per_rank=FEAT * 2,
                    single_packet=False, queue_num=gq[0])

            def emit_S(dloc_t, t0, nt, tag):
                S = spool.tile([128, nt * 128], f16, tag=tag)
                in0 = dloc_t[:, t0:t0 + nt].unsqueeze(2).broadcast_to(
                    (128, nt, 128))
                in1 = iota[:].unsqueeze(1).broadcast_to((128, nt, 128))
                nc.vector.tensor_tensor(
                    S[:].rearrange("p (t d) -> p t d", t=nt), in0, in1,
                    mybir.AluOpType.is_equal)
                return S

            glob = 0
            for rep in range(reps):
                ag_a = ag_b = None
                for l in range(n_layers):
                    fo = fos[l]
                    A = tblA[glob % 2]
                    glob += 1
                    if l == 0:
                        load_tbl(A, xa_in, rka)
                        load_tbl(tblB, xb_in, rkb)
                    else:
                        if model:
                            ta = dram.tile([NRA, FEAT], f16, tag="ta")
                            tb = dram.tile([NRB, FEAT], f16, tag="tb")
                            nc.sync.dma_start(out=ta[0:rka * 128, :], in_=ag_a[:])
                            nc.sync.dma_start(out=tb[0:rkb * 128, :], in_=ag_b[:])
                        else:
                            ta = dram_sh.tile([NRA, FEAT], f16, tag="ta",
                                              addr_space="Shared")
                            tb = dram_sh.tile([NRB, FEAT], f16, tag="tb",
                                              addr_space="Shared")
                            nc.gpsimd.collective_compute(
                                "AllGather", mybir.AluOpType.bypass,
                                replica_groups=rg,
                                ins=[ag_a[:].opt()], outs=[ta[:].opt()])
                            nc.gpsimd.collective_compute(
                                "AllGather", mybir.AluOpType.bypass,
                                replica_groups=rg,
                                ins=[ag_b[:].opt()], outs=[tb[:].opt()])
                        load_tbl(A, ta, rka)
                        load_tbl(tblB, tb, rkb)

                    if l + 1 < n_layers:
                        ag_a = dram.tile([rka * 128, FEAT], f16, tag="ag_a")
                        ag_b = dram.tile([rkb * 128, FEAT], f16, tag="ag_b")
                        ag_a_v = ag_a[:].rearrange("(p r) f -> p r f", p=128)
                        ag_b_v = ag_b[:].rearrange("(p r) f -> p r f", p=128)

                    for chunk in chunks:
                        c0 = chunk[0]
                        ctlo = int(sum(T_lo[b] for b in chunk))
                        cthi = int(sum(T_hi[b] for b in chunk))
                        M_lo = M_hi = S_lo = S_hi = None
                        if ctlo:
                            M_lo = mbuf.tile([128, ctlo * 128], f16, tag="Mlo")
                            emit_gather(M_lo, A, idx_lo, int(cum_lo[c0]), ctlo)
                            S_lo = emit_S(dloc_lo, int(cum_lo[c0]), ctlo, "Slo")
                        if cthi:
                            M_hi = mbuf.tile([128, cthi * 128], f16, tag="Mhi")
                            emit_gather(M_hi, tblB, idx_hi, int(cum_hi[c0]), cthi)
                            S_hi = emit_S(dloc_hi, int(cum_hi[c0]), cthi, "Shi")

                        # pass 1: transform every tile of the chunk
                        tiles = []   # (block, S, slice) in agg order
                        Ps_list = []
                        for b in chunk:
                            for h, (T, cum, M, S) in enumerate((
                                    (T_lo, cum_lo, M_lo, S_lo),
                                    (T_hi, cum_hi, M_hi, S_hi))):
                                for k in range(int(T[b])):
                                    sl = int(cum[b] - cum[c0]) + k
                                    P = psum_p.tile([128, fo], f32)
                                    nc.tensor.matmul(
                                        P[:], M[:, sl * 128:(sl + 1) * 128],
                                        Wt[l][:], start=True, stop=True)
                                    Ps = pspool.tile([128, fo], f16, tag="Ps")
                                    if len(Ps_list) % 2 == 0:
                                        nc.scalar.copy(Ps[:], P[:])
                                    else:
                                        nc.vector.tensor_scalar_mul(
                                            Ps[:], P[:], 1.0)
                                    tiles.append((b, S, sl))
                                    Ps_list.append(Ps)

                        # pass 2: aggregate per block
                        ti = 0
                        for b in chunk:
                            ntb = int(T_lo[b]) + int(T_hi[b])
                            agg = psum_agg.tile([128, fo], f32)
                            row = 0 if l + 1 < n_layers else 32
                            nc.tensor.matmul(
                                agg[:],
                                rdeg[row:row + 1, b * 128:(b + 1) * 128],
                                bt[l][row:row + 1, :],
                                start=True, stop=(ntb == 0))
                            for j in range(ntb):
                                _, S, sl = tiles[ti]
                                nc.tensor.matmul(
                                    agg[:], S[:, sl * 128:(sl + 1) * 128],
                                    Ps_list[ti][:], start=False,
                                    stop=(j == ntb - 1))
                                ti += 1
                            r0 = b * 128
                            rows = min(128, S_pc - r0)
                            if l + 1 < n_layers:
                                hb = sb.tile([128, fo], f16, tag="hb")
                                nc.scalar.activation(
                                    hb[:], agg[:],
                                    mybir.ActivationFunctionType.Relu,
                                    scale=dinv2_blk[:, b:b + 1])
                                if b < ABLK:
                                    nc.sync.dma_start(
                                        out=ag_a_v[:, b, :], in_=hb[:])
                                else:
                                    nc.sync.dma_start(
                                        out=ag_b_v[0:rows, b - ABLK, :],
                                        in_=hb[0:rows, :])
                            else:
                                yb = sb.tile([128, fo], f32, tag="yb")
                                nc.scalar.activation(
                                    yb[:], agg[:],
                                    mybir.ActivationFunctionType.Copy,
                                    scale=dinv_blk[:, b:b + 1])
                                nc.sync.dma_start(
                                    out=y_out[r0:r0 + rows, :],
                                    in_=yb[0:rows, :])
    if compile:
        nc.compile()
    return nc


# ---------------------------------------------------------------- entry point

_CACHE = {}


def kernel(x, edge_index, W1, b1, W2, b2, W3, b3):
    import sys
    if "/opt/trn_rl_repo" not in sys.path:
        sys.path.insert(0, "/opt/trn_rl_repo")
    from concourse import bass_utils

    x = np.asarray(x)
    edge_index = np.asarray(edge_index)
    Ws = [np.asarray(W1), np.asarray(W2), np.asarray(W3)]
    bs = [np.asarray(b1), np.asarray(b2), np.asarray(b3)]
    n = x.shape[0]

    key = (n, edge_index.shape[1])
    if key in _CACHE and np.array_equal(_CACHE[key][0], edge_index):
        _, sched, per_core, nc = _CACHE[key]
    else:
        sched, per_core = preprocess(edge_index, n, N_CORES)
        nc = build_nc(sched, fos=(W1.shape[1], W2.shape[1], W3.shape[1]))
        _CACHE[key] = (edge_index.copy(), sched, per_core, nc)

    in_maps = make_inputs(sched, per_core, x, Ws, bs)
    res = bass_utils.run_bass_kernel_spmd(nc, in_maps,
                                          core_ids=list(range(N_CORES)))
    out = np.concatenate([res.results[r]["y"] for r in range(N_CORES)], axis=0)
    return out.astype(np.float32)


# revision 10
# speedup vs baseline: 1.0505x; 1.0505x over previous
"""3-layer GCN (PyG GCNConv semantics) on 8 Trainium2 NeuronCores.

Contract: kernel(**inputs) takes the FULL inputs (x [50000,128] f32,
edge_index [2,800000] int, W1/b1/W2/b2/W3/b3) and returns the FULL
output [50000, 64] f32.

Design: nodes are partitioned across the 8 cores by destination (6250
rows each).  Per layer every core keeps the FULL node-feature table
SBUF-resident (50000 x 128 fp16 ~ 12.8MB) in a token layout and expands
per-edge messages with SBUF-source transposed dma_gather (~1ns/desc vs
~4.7ns/desc for random HBM reads).  Norms are folded into the table
rows (t_u = dinv_u * h_u) so per-edge weights vanish: for edge (s,d)
msg = dinv_d * t_s, and self-loops are ordinary (u,u) edges.  Per edge
tile the fp16 PE computes P[e,fo] = M^T[f,e]^T W[f,fo] (fused
transpose+transform of the gathered strip) and agg[d,fo] += S01[e,d]^T
P[e,fo] with a batched 0/1 one-hot S built by a single broadcast DVE
is_equal per chunk.  Bias enters as a rank-1 matmul (deg[d] (x) b[fo])
into the same PSUM bank; the block epilogue is one scalar-engine
activation Relu(scale=dinv^2) producing the next layer's table rows.
Between layers the 0.8MB per-core shard is AllGathered and the table
re-loaded into SBUF with fully contiguous 6.4KB strides; two A-region
table buffers alternate so the A reload overlaps the B half's compute.
The table is split in two regions (A: local rows [0,3200) of each
core, B: the rest padded to 3072) to keep gather indices in int16.
"""

import numpy as np

FEAT = 128
N_CORES = 8
ABLK = 17            # blocks per core in table region A
CHUNK_BLOCKS = 2


# ---------------------------------------------------------------- host side

def preprocess(edge_index: np.ndarray, n_nodes: int, n_cores: int = N_CORES,
               chunk_blocks: int = CHUNK_BLOCKS):
    src = np.asarray(edge_index[0], dtype=np.int64)
    dst = np.asarray(edge_index[1], dtype=np.int64)
    deg = (np.bincount(dst, minlength=n_nodes) + 1).astype(np.float64)
    dinv = 1.0 / np.sqrt(deg)

    loops = np.arange(n_nodes, dtype=np.int64)
    s = np.concatenate([src, loops])
    d = np.concatenate([dst, loops])

    S_pc = n_nodes // n_cores
    assert S_pc * n_cores == n_nodes
    nblocks = (S_pc + 127) // 128
    RA = ABLK * 128                       # 3200 local rows in region A
    rka = ABLK                            # ranks per core, region A
    rkb = nblocks - ABLK                  # 24 -> 3072 padded local rows in B
    RB = S_pc - RA                        # 3050 real local rows in B
    assert n_cores * rka * 128 <= 32768 and n_cores * rkb * 128 <= 32768

    cs, ls = s // S_pc, s % S_pc
    in_a = ls < RA
    lb = ls - RA
    idx = np.where(in_a,
                   (cs * rka + (ls >> 7)) * 128 + (ls & 127),
                   (cs * rkb + (lb >> 7)) * 128 + (lb & 127))
    half = (~in_a).astype(np.int64)
    core = d // S_pc
    dl = d - core * S_pc
    blk = dl >> 7
    dloc = (dl & 127).astype(np.int64)

    counts = np.zeros((n_cores, nblocks, 2), dtype=np.int64)
    np.add.at(counts, (core, blk, half), 1)
    T = -(-counts.max(axis=0) // 128)     # [nblocks, 2] tiles per block/half
    T_lo, T_hi = T[:, 0].astype(int), T[:, 1].astype(int)
    NT_lo, NT_hi = int(T_lo.sum()), int(T_hi.sum())
    cum_lo = np.concatenate([[0], np.cumsum(T_lo)]).astype(int)
    cum_hi = np.concatenate([[0], np.cumsum(T_hi)]).astype(int)

    order = np.lexsort((idx, half, blk, core))
    idx_o, dl_o = idx[order], dloc[order]
    key = (core[order] * nblocks + blk[order]) * 2 + half[order]
    bounds = np.searchsorted(key, np.arange(n_cores * nblocks * 2 + 1))

    def wrap(a):  # [n] int -> [128, n//16] i16 wrap layout
        n = len(a)
        if n == 0:
            return np.zeros((128, 1), dtype=np.int16)
        w = a.reshape(n // 16, 16).T
        return np.tile(w, (8, 1)).copy()

    per_core = []
    for r in range(n_cores):
        idx_lo = np.zeros(128 * max(NT_lo, 1), dtype=np.int16)
        idx_hi = np.zeros(128 * max(NT_hi, 1), dtype=np.int16)
        dloc_lo = np.full((128, max(NT_lo, 1)), -1.0, dtype=np.float16)
        dloc_hi = np.full((128, max(NT_hi, 1)), -1.0, dtype=np.float16)
        for b in range(nblocks):
            for h in range(2):
                k = (r * nblocks + b) * 2 + h
                lo_, hi_ = bounds[k], bounds[k + 1]
                cnt = hi_ - lo_
                t0 = cum_lo[b] if h == 0 else cum_hi[b]
                iarr = idx_lo if h == 0 else idx_hi
                darr = dloc_lo if h == 0 else dloc_hi
                iarr[128 * t0: 128 * t0 + cnt] = idx_o[lo_:hi_].astype(np.int16)
                e = np.arange(cnt)
                darr[e % 128, t0 + e // 128] = dl_o[lo_:hi_]

        # per-block per-partition scales (pad partitions of last block -> 0)
        p_all = np.arange(nblocks * 128)
        ok = p_all < S_pc
        dv = np.zeros(nblocks * 128, dtype=np.float64)
        dv[ok] = dinv[r * S_pc + p_all[ok]]
        dinv_blk = np.ascontiguousarray(
            dv.reshape(nblocks, 128).T.astype(np.float32))
        dinv2_blk = np.ascontiguousarray(
            (dv ** 2).reshape(nblocks, 128).T.astype(np.float32))
        # rank-1 bias rows: row0 = deg (layers 0,1: 1/dinv^2), row1 = sqrt(deg)
        rdeg = np.zeros((128, nblocks * 128), dtype=np.float16)
        dgv = np.zeros(nblocks * 128, dtype=np.float64)
        dgv[ok] = deg[r * S_pc + p_all[ok]]
        rdeg[0, :] = dgv.astype(np.float16)
        rdeg[32, :] = np.sqrt(dgv).astype(np.float16)
        per_core.append(dict(
            idx_lo=wrap(idx_lo), idx_hi=wrap(idx_hi),
            dloc_lo=dloc_lo, dloc_hi=dloc_hi,
            dinv_blk=dinv_blk, dinv2_blk=dinv2_blk, rdeg=rdeg,
        ))

    # table build maps: flat A row (c,p,r) -> node id; B likewise with pad mask
    c = np.arange(n_cores)[:, None, None]
    p = np.arange(128)[None, :, None]
    ra = np.arange(rka)[None, None, :]
    amap = (c * S_pc + ra * 128 + p).reshape(-1)
    rb = np.arange(rkb)[None, None, :]
    brow = RA + rb * 128 + p
    bmask = (brow < S_pc)
    bmap = (c * S_pc + np.minimum(brow, S_pc - 1)).reshape(-1)
    bmask = np.broadcast_to(bmask, (n_cores, 128, rkb)).reshape(-1)

    chunks = [list(range(cc, min(cc + chunk_blocks, nblocks)))
              for cc in range(0, nblocks, chunk_blocks)]
    sched = dict(
        n_nodes=n_nodes, n_cores=n_cores, S_pc=S_pc, nblocks=nblocks,
        rka=rka, rkb=rkb, RA=RA, RB=RB,
        T_lo=T_lo, T_hi=T_hi, cum_lo=cum_lo, cum_hi=cum_hi,
        n_tiles_lo=NT_lo, n_tiles_hi=NT_hi, chunks=chunks,
        dinv=dinv.astype(np.float32), amap=amap, bmap=bmap, bmask=bmask,
    )
    return sched, per_core


def make_inputs(sched, per_core, x, Ws, bs):
    n_cores = sched["n_cores"]
    dinv = sched["dinv"]
    t0 = (np.asarray(x, np.float32) * dinv[:, None]).astype(np.float16)
    xa = np.ascontiguousarray(t0[sched["amap"]])
    xb = t0[sched["bmap"]].copy()
    xb[~sched["bmask"]] = 0
    iota = np.tile(np.arange(128, dtype=np.float16)[None, :], (128, 1))
    in_maps = []
    for r in range(n_cores):
        m = dict(
            xa=xa, xb=xb, iota=iota,
            idx_lo=per_core[r]["idx_lo"], idx_hi=per_core[r]["idx_hi"],
            dloc_lo=per_core[r]["dloc_lo"], dloc_hi=per_core[r]["dloc_hi"],
            dinv_blk=per_core[r]["dinv_blk"],
            dinv2_blk=per_core[r]["dinv2_blk"],
            rdeg=per_core[r]["rdeg"],
        )
        for i, (W, b) in enumerate(zip(Ws, bs)):
            m[f"W{i}"] = np.asarray(W).astype(np.float16)
            m[f"b{i}"] = np.asarray(b, dtype=np.float16)[None, :]
        in_maps.append(m)
    return in_maps


# ---------------------------------------------------------------- device side

def build_nc(sched, fos=(128, 128, 64), n_cores=None, model=False,
             compile=True, reps=1, swdge_queues=1, has_bias=False,
             chunk_blocks=None, gmax=16):
    import concourse.bacc as bacc
    import concourse.tile as tile
    import concourse.mybir as mybir

    f16, f32, i16 = mybir.dt.float16, mybir.dt.float32, mybir.dt.int16
    N, S_pc = sched["n_nodes"], sched["S_pc"]
    nblocks, rka, rkb = sched["nblocks"], sched["rka"], sched["rkb"]
    T_lo, T_hi = sched["T_lo"], sched["T_hi"]
    cum_lo, cum_hi = sched["cum_lo"], sched["cum_hi"]
    NT_lo, NT_hi = sched["n_tiles_lo"], sched["n_tiles_hi"]
    chunks = sched["chunks"]
    n_cores = n_cores or sched["n_cores"]
    n_layers = len(fos)
    NRA, NRB = n_cores * rka * 128, n_cores * rkb * 128   # table rows

    nc = bacc.Bacc("TRN2", target_bir_lowering=False, debug=False,
                   num_devices=n_cores, num_swdge_queues=swdge_queues)

    xa_in = nc.dram_tensor("xa", [NRA, FEAT], f16, kind="ExternalInput")
    xb_in = nc.dram_tensor("xb", [NRB, FEAT], f16, kind="ExternalInput")
    iota_in = nc.dram_tensor("iota", [128, 128], f16, kind="ExternalInput")
    idx_lo_in = nc.dram_tensor("idx_lo", [128, max(NT_lo, 1) * 8], i16, kind="ExternalInput")
    idx_hi_in = nc.dram_tensor("idx_hi", [128, max(NT_hi, 1) * 8], i16, kind="ExternalInput")
    dloc_lo_in = nc.dram_tensor("dloc_lo", [128, max(NT_lo, 1)], f16, kind="ExternalInput")
    dloc_hi_in = nc.dram_tensor("dloc_hi", [128, max(NT_hi, 1)], f16, kind="ExternalInput")
    dinv_in = nc.dram_tensor("dinv_blk", [128, nblocks], f32, kind="ExternalInput")
    dinv2_in = nc.dram_tensor("dinv2_blk", [128, nblocks], f32, kind="ExternalInput")
    if has_bias:
        rdeg_in = nc.dram_tensor("rdeg", [128, nblocks * 128], f16, kind="ExternalInput")
    W_in = [nc.dram_tensor(f"W{i}", [FEAT, fos[i]], f16, kind="ExternalInput")
            for i in range(n_layers)]
    b_in = [nc.dram_tensor(f"b{i}", [1, fos[i]], f16, kind="ExternalInput")
            for i in range(n_layers)]
    y_out = nc.dram_tensor("y", [S_pc, fos[-1]], f32, kind="ExternalOutput")

    rg = [list(range(n_cores))]

    with tile.TileContext(nc) as tc:
        with (
            tc.tile_pool(name="const", bufs=1) as cpool,
            tc.tile_pool(name="sb", bufs=3) as sb,
            tc.tile_pool(name="ps", bufs=6) as pspool,
            tc.tile_pool(name="mbuf", bufs=2) as mbuf,
            tc.tile_pool(name="spool", bufs=2) as spool,
            tc.tile_pool(name="psum_p", bufs=6, space="PSUM") as psum_p,
            tc.tile_pool(name="psum_agg", bufs=2, space="PSUM") as psum_agg,
            tc.tile_pool(name="dram", bufs=2, space="DRAM") as dram,
            tc.tile_pool(name="dram_sh", bufs=2, space="DRAM") as dram_sh,
        ):
            # --- constants
            iota = cpool.tile([128, 128], f16)
            nc.sync.dma_start(out=iota[:], in_=iota_in[:])
            idx_lo = cpool.tile([128, max(NT_lo, 1) * 8], i16)
            nc.sync.dma_start(out=idx_lo[:], in_=idx_lo_in[:])
            idx_hi = cpool.tile([128, max(NT_hi, 1) * 8], i16)
            nc.sync.dma_start(out=idx_hi[:], in_=idx_hi_in[:])
            dloc_lo = cpool.tile([128, max(NT_lo, 1)], f16)
            nc.sync.dma_start(out=dloc_lo[:], in_=dloc_lo_in[:])
            dloc_hi = cpool.tile([128, max(NT_hi, 1)], f16)
            nc.sync.dma_start(out=dloc_hi[:], in_=dloc_hi_in[:])
            dinv_blk = cpool.tile([128, nblocks], f32)
            nc.sync.dma_start(out=dinv_blk[:], in_=dinv_in[:])
            dinv2_blk = cpool.tile([128, nblocks], f32)
            nc.sync.dma_start(out=dinv2_blk[:], in_=dinv2_in[:])
            if has_bias:
                rdeg = cpool.tile([128, nblocks * 128], f16)
                nc.sync.dma_start(out=rdeg[:], in_=rdeg_in[:])
            Wt, bt = [], []
            for i in range(n_layers):
                w = cpool.tile([FEAT, fos[i]], f16, tag=f"W{i}")
                nc.sync.dma_start(out=w[:], in_=W_in[i][:])
                Wt.append(w)
                if has_bias:
                    b = cpool.tile([128, fos[i]], f16, tag=f"b{i}")
                    nc.vector.memset(b[:], 0.0)
                    nc.sync.dma_start(out=b[0:1, :], in_=b_in[i][:])
                    nc.sync.dma_start(out=b[32:33, :], in_=b_in[i][:])
                    bt.append(b)

            # --- SBUF tables (token layout: node -> partition tok, rank slot)
            tblA = cpool.tile([128, n_cores * rka * FEAT], f16, tag="tblA")
            tblB = cpool.tile([128, n_cores * rkb * FEAT], f16, tag="tblB")

            def load_tbl(dst, src, rk):
                # src flat (c,p,r) rows of 256B -> dst[p, (c r)*256B]
                nc.sync.dma_start(
                    out=dst[:].rearrange("p (c r f) -> p c r f",
                                         c=n_cores, r=rk),
                    in_=src[:].rearrange("(c p r) f -> p c r f",
                                         p=128, r=rk))

            def emit_gathers(M, tbl, idx_sb, t0, nt):
                # split into pieces of <= gmax tiles (2048 idxs each)
                npieces = -(-nt // gmax)
                step = -(-nt // npieces)
                for s0 in range(0, nt, step):
                    sn = min(step, nt - s0)
                    nc.gpsimd.dma_gather(
                        out_ap=M[:, s0 * 128:(s0 + sn) * 128].rearrange(
                            "p (o e) -> p o e", o=1),
                        in_ap=tbl[:],
                        idxs_ap=idx_sb[:, (t0 + s0) * 8:(t0 + s0 + sn) * 8],
                        num_idxs=128 * sn, num_idxs_reg=128 * sn,
                        elem_size=FEAT, transpose=True,
                        sbuf_tokens_per_rank=128,
                        sbuf_free_dim_per_rank=FEAT * 2,
                        single_packet=False, queue_num=0)

            def emit_S(dloc_t, t0, nt, tag):
                S = spool.tile([128, nt * 128], f16, tag=tag, name=tag)
                in0 = dloc_t[:, t0:t0 + nt].unsqueeze(2).broadcast_to(
                    (128, nt, 128))
                in1 = iota[:].unsqueeze(1).broadcast_to((128, nt, 128))
                nc.vector.tensor_tensor(
                    S[:].rearrange("p (t d) -> p t d", t=nt), in0, in1,
                    mybir.AluOpType.is_equal)
                return S

            GRP = 4
            for rep in range(reps):
                ag_a = ag_b = None
                for l in range(n_layers):
                    fo = fos[l]
                    if l == 0:
                        load_tbl(tblA, xa_in, rka)
                        load_tbl(tblB, xb_in, rkb)
                    else:
                        if model:
                            ta = dram.tile([NRA, FEAT], f16, tag="ta")
                            tb = dram.tile([NRB, FEAT], f16, tag="tb")
                            nc.sync.dma_start(out=ta[0:rka * 128, :], in_=ag_a[:])
                            nc.sync.dma_start(out=tb[0:rkb * 128, :], in_=ag_b[:])
                        else:
                            ta = dram_sh.tile([NRA, FEAT], f16, tag="ta",
                                              addr_space="Shared")
                            tb = dram_sh.tile([NRB, FEAT], f16, tag="tb",
                                              addr_space="Shared")
                            nc.gpsimd.collective_compute(
                                "AllGather", mybir.AluOpType.bypass,
                                replica_groups=rg,
                                ins=[ag_a[:].opt()], outs=[ta[:].opt()])
                            nc.gpsimd.collective_compute(
                                "AllGather", mybir.AluOpType.bypass,
                                replica_groups=rg,
                                ins=[ag_b[:].opt()], outs=[tb[:].opt()])
                        load_tbl(tblA, ta, rka)
                        load_tbl(tblB, tb, rkb)

                    if l + 1 < n_layers:
                        ag_a = dram.tile([rka * 128, FEAT], f16, tag="ag_a")
                        ag_b = dram.tile([rkb * 128, FEAT], f16, tag="ag_b")
                        ag_a_v = ag_a[:].rearrange("(p r) f -> p r f", p=128)
                        ag_b_v = ag_b[:].rearrange("(p r) f -> p r f", p=128)

                    ci = 0
                    for chunk in chunks:
                        c0 = chunk[0]
                        ctlo = int(sum(T_lo[b] for b in chunk))
                        cthi = int(sum(T_hi[b] for b in chunk))
                        M_lo = M_hi = S_lo = S_hi = None
                        if ctlo:
                            M_lo = mbuf.tile([128, ctlo * 128], f16, tag="Mlo")
                            emit_gathers(M_lo, tblA, idx_lo, int(cum_lo[c0]), ctlo)
                            S_lo = emit_S(dloc_lo, int(cum_lo[c0]), ctlo, "Slo")
                        if cthi:
                            M_hi = mbuf.tile([128, cthi * 128], f16, tag="Mhi")
                            emit_gathers(M_hi, tblB, idx_hi, int(cum_hi[c0]), cthi)
                            S_hi = emit_S(dloc_hi, int(cum_hi[c0]), cthi, "Shi")

                        # flat tile list in block order
                        flat = []    # (M, sl)
                        marks = []   # number of tiles per block
                        for b in chunk:
                            nb0 = len(flat)
                            for (T, cum, M, S) in ((T_lo, cum_lo, M_lo, S_lo),
                                                   (T_hi, cum_hi, M_hi, S_hi)):
                                for k in range(int(T[b])):
                                    sl = int(cum[b] - cum[c0]) + k
                                    flat.append((M, sl, S))
                            marks.append(len(flat) - nb0)

                        # pass 1: transform, 4 tiles per PSUM bank, one
                        # batched PSUM->SBUF fp16 copy per group
                        tinfo = []   # (S, sl, Ps, q)
                        for g0 in range(0, len(flat), GRP):
                            grp = flat[g0:g0 + GRP]
                            P = psum_p.tile([128, GRP * fo], f32)
                            for q, (M, sl, S) in enumerate(grp):
                                nc.tensor.matmul(
                                    P[:, q * fo:(q + 1) * fo],
                                    M[:, sl * 128:(sl + 1) * 128],
                                    Wt[l][:], start=True, stop=True,
                                    skip_group_check=True)
                            Ps = pspool.tile([128, GRP * fo], f16, tag="Ps")
                            w = len(grp) * fo
                            if (g0 // GRP) % 2 == 0:
                                nc.scalar.copy(Ps[:, 0:w], P[:, 0:w])
                            else:
                                nc.vector.tensor_scalar_mul(
                                    Ps[:, 0:w], P[:, 0:w], 1.0)
                            for q, (M, sl, S) in enumerate(grp):
                                tinfo.append((S, sl, Ps, q))

                        # pass 2: aggregate per block + epilogue
                        ti = 0
                        for bi, b in enumerate(chunk):
                            ntb = marks[bi]
                            agg = psum_agg.tile([128, fo], f32)
                            row = 0 if l + 1 < n_layers else 32
                            if has_bias:
                                nc.tensor.matmul(
                                    agg[:],
                                    rdeg[row:row + 1, b * 128:(b + 1) * 128],
                                    bt[l][row:row + 1, :],
                                    start=True, stop=False,
                                    skip_group_check=True)
                            for j in range(ntb):
                                S, sl, Ps, q = tinfo[ti]
                                nc.tensor.matmul(
                                    agg[:], S[:, sl * 128:(sl + 1) * 128],
                                    Ps[:, q * fo:(q + 1) * fo],
                                    start=(j == 0 and not has_bias),
                                    stop=(j == ntb - 1),
                                    skip_group_check=True)
                                ti += 1
                            r0 = b * 128
                            rows = min(128, S_pc - r0)
                            if l + 1 < n_layers:
                                hb = sb.tile([128, fo], f16, tag="hb")
                                nc.scalar.activation(
                                    hb[:], agg[:],
                                    mybir.ActivationFunctionType.Relu,
                                    scale=dinv2_blk[:, b:b + 1])
                                if b < ABLK:
                                    nc.sync.dma_start(
                                        out=ag_a_v[:, b, :], in_=hb[:])
                                else:
                                    nc.sync.dma_start(
                                        out=ag_b_v[0:rows, b - ABLK, :],
                                        in_=hb[0:rows, :])
                            else:
                                yb = sb.tile([128, fo], f32, tag="yb")
                                nc.scalar.activation(
                                    yb[:], agg[:],
                                    mybir.ActivationFunctionType.Copy,
                                    scale=dinv_blk[:, b:b + 1])
                                nc.sync.dma_start(
                                    out=y_out[r0:r0 + rows, :],
                                    in_=yb[0:rows, :])
                        ci += 1
    if compile:
        nc.compile()
    return nc


# ---------------------------------------------------------------- entry point

_CACHE = {}


def kernel(x, edge_index, W1, b1, W2, b2, W3, b3):
    import sys
    if "/opt/trn_rl_repo" not in sys.path:
        sys.path.insert(0, "/opt/trn_rl_repo")
    from concourse import bass_utils

    x = np.asarray(x)
    edge_index = np.asarray(edge_index)
    Ws = [np.asarray(W1), np.asarray(W2), np.asarray(W3)]
    bs = [np.asarray(b1), np.asarray(b2), np.asarray(b3)]
    n = x.shape[0]
    has_bias = any(np.any(np.asarray(b) != 0) for b in bs)

    key = (n, edge_index.shape[1], has_bias)
    if key in _CACHE and np.array_equal(_CACHE[key][0], edge_index):
        _, sched, per_core, nc = _CACHE[key]
    else:
        sched, per_core = preprocess(edge_index, n, N_CORES)
        nc = build_nc(sched, fos=(W1.shape[1], W2.shape[1], W3.shape[1]),
                      has_bias=has_bias)
        _CACHE[key] = (edge_index.copy(), sched, per_core, nc)

    in_maps = make_inputs(sched, per_core, x, Ws, bs)
    if not has_bias:
        for m in in_maps:
            m.pop("rdeg", None)
    res = bass_utils.run_bass_kernel_spmd(nc, in_maps,
                                          core_ids=list(range(N_CORES)))
    out = np.concatenate([res.results[r]["y"] for r in range(N_CORES)], axis=0)
    return out.astype(np.float32)


# revision 16
# speedup vs baseline: 2.4962x; 2.3762x over previous
"""3-layer GCN (PyG GCNConv semantics) on 8 Trainium2 NeuronCores.

Contract: kernel(**inputs) takes the FULL inputs (x [50000,128] f32,
edge_index [2,800000] int, W1/b1/W2/b2/W3/b3) and returns the FULL
output [50000, 64] f32.

Design: nodes are partitioned across the 8 cores by destination (6250
rows each).  Edge norms are folded into the table rows (t_u = dinv_u *
h_u) so per-edge weights vanish (msg = dinv_d * t_s) and self-loops are
ordinary (u,u) edges.  Per-edge message expansion is split across two
independent gather mechanisms that run concurrently:

  * region A (local rows [0, ABLK*128) of each core): the fp16 table
    lives SBUF-resident in a token layout; SBUF-source transposed
    dma_gather on SWDGE queue 0 produces f-major strips M^T[f,e]; the
    PE fuses transpose+transform per tile (P[e,fo] = M^T.T @ W) and
    aggregates agg[d,fo] += S01[e,d]^T P[e,fo].  Transposed gathers
    are limited by a shared RX transpose unit (~6.5ns/desc) and race
    when spread across queues, so they stay serialized on one queue.
  * region B (the rest): plain non-transposed SWDGE gathers on queues
    1..3 read 256B rows straight from the chip-shared HBM table
    (the AllGather output); tiles are e-major so they aggregate
    directly (aggB[f,d] += M[e,f]^T S01[e,d]) with one per-block
    transform matmul aggB^T @ W into the same PSUM agg bank.

S01 one-hot strips are built in one broadcast DVE is_equal per chunk
half.  The block epilogue is a single scalar-engine activation
Relu(scale=dinv^2) producing the next layer's table rows.  Between
layers the per-core shard is AllGathered; region A is re-loaded into
one of two alternating SBUF buffers with contiguous 6.4KB strides (so
the reload overlaps the other half's compute), region B is gathered
from HBM directly.  Region sizes keep all gather indices in int16.
"""

import numpy as np

FEAT = 128
N_CORES = 8
ABLK = 17            # blocks per core in table region A (17..32)
CHUNK_BLOCKS = 2


# ---------------------------------------------------------------- host side

def preprocess(edge_index: np.ndarray, n_nodes: int, n_cores: int = N_CORES,
               chunk_blocks: int = CHUNK_BLOCKS, ablk: int = ABLK):
    src = np.asarray(edge_index[0], dtype=np.int64)
    dst = np.asarray(edge_index[1], dtype=np.int64)
    deg = (np.bincount(dst, minlength=n_nodes) + 1).astype(np.float64)
    dinv = 1.0 / np.sqrt(deg)

    loops = np.arange(n_nodes, dtype=np.int64)
    s = np.concatenate([src, loops])
    d = np.concatenate([dst, loops])

    S_pc = n_nodes // n_cores
    assert S_pc * n_cores == n_nodes
    nblocks = (S_pc + 127) // 128
    rka = ablk                            # ranks per core, region A
    rkb = nblocks - ablk                  # ranks per core, region B (padded)
    RA = rka * 128
    assert n_cores * rka * 128 <= 32768 and n_cores * rkb * 128 <= 32768

    cs, ls = s // S_pc, s % S_pc
    in_a = ls < RA
    lb = ls - RA
    # A: SBUF token layout index; B: plain row number in the padded table
    idx = np.where(in_a,
                   (cs * rka + (ls >> 7)) * 128 + (ls & 127),
                   cs * (rkb * 128) + lb)
    half = (~in_a).astype(np.int64)
    core = d // S_pc
    dl = d - core * S_pc
    blk = dl >> 7
    dloc = (dl & 127).astype(np.int64)

    counts = np.zeros((n_cores, nblocks, 2), dtype=np.int64)
    np.add.at(counts, (core, blk, half), 1)
    T = -(-counts.max(axis=0) // 128)     # [nblocks, 2] tiles per block/half
    T_lo, T_hi = T[:, 0].astype(int), T[:, 1].astype(int)
    NT_lo, NT_hi = int(T_lo.sum()), int(T_hi.sum())
    cum_lo = np.concatenate([[0], np.cumsum(T_lo)]).astype(int)
    cum_hi = np.concatenate([[0], np.cumsum(T_hi)]).astype(int)

    order = np.lexsort((idx, half, blk, core))
    idx_o, dl_o = idx[order], dloc[order]
    key = (core[order] * nblocks + blk[order]) * 2 + half[order]
    bounds = np.searchsorted(key, np.arange(n_cores * nblocks * 2 + 1))

    def wrap(a):  # [n] int -> [128, n//16] i16 wrap layout
        n = len(a)
        if n == 0:
            return np.zeros((128, 1), dtype=np.int16)
        w = a.reshape(n // 16, 16).T
        return np.tile(w, (8, 1)).copy()

    per_core = []
    for r in range(n_cores):
        idx_lo = np.zeros(128 * max(NT_lo, 1), dtype=np.int16)
        idx_hi = np.zeros(128 * max(NT_hi, 1), dtype=np.int16)
        dloc_lo = np.full((128, max(NT_lo, 1)), -1.0, dtype=np.float16)
        dloc_hi = np.full((128, max(NT_hi, 1)), -1.0, dtype=np.float16)
        for b in range(nblocks):
            for h in range(2):
                k = (r * nblocks + b) * 2 + h
                lo_, hi_ = bounds[k], bounds[k + 1]
                cnt = hi_ - lo_
                t0 = cum_lo[b] if h == 0 else cum_hi[b]
                iarr = idx_lo if h == 0 else idx_hi
                darr = dloc_lo if h == 0 else dloc_hi
                iarr[128 * t0: 128 * t0 + cnt] = idx_o[lo_:hi_].astype(np.int16)
                e = np.arange(cnt)
                darr[e % 128, t0 + e // 128] = dl_o[lo_:hi_]

        # per-block per-partition scales (pad partitions of last block -> 0)
        p_all = np.arange(nblocks * 128)
        ok = p_all < S_pc
        dv = np.zeros(nblocks * 128, dtype=np.float64)
        dv[ok] = dinv[r * S_pc + p_all[ok]]
        dinv_blk = np.ascontiguousarray(
            dv.reshape(nblocks, 128).T.astype(np.float32))
        dinv2_blk = np.ascontiguousarray(
            (dv ** 2).reshape(nblocks, 128).T.astype(np.float32))
        # rank-1 bias rows: row0 = deg (layers 0,1), row32 = sqrt(deg)
        rdeg = np.zeros((128, nblocks * 128), dtype=np.float16)
        dgv = np.zeros(nblocks * 128, dtype=np.float64)
        dgv[ok] = deg[r * S_pc + p_all[ok]]
        rdeg[0, :] = dgv.astype(np.float16)
        rdeg[32, :] = np.sqrt(dgv).astype(np.float16)
        per_core.append(dict(
            idx_lo=wrap(idx_lo), idx_hi=wrap(idx_hi),
            dloc_lo=dloc_lo, dloc_hi=dloc_hi,
            dinv_blk=dinv_blk, dinv2_blk=dinv2_blk, rdeg=rdeg,
        ))

    # region A build map: flat (c,p,r) row -> node id
    c = np.arange(n_cores)[:, None, None]
    p = np.arange(128)[None, :, None]
    ra = np.arange(rka)[None, None, :]
    amap = (c * S_pc + ra * 128 + p).reshape(-1)
    # region B: plain padded rows (c-major)
    lbv = np.arange(rkb * 128)[None, :]
    brow = RA + lbv
    bmask = (brow < S_pc)
    bmap = (np.arange(n_cores)[:, None] * S_pc
            + np.minimum(brow, S_pc - 1)).reshape(-1)
    bmask = np.broadcast_to(bmask, (n_cores, rkb * 128)).reshape(-1)

    chunks = [list(range(cc, min(cc + chunk_blocks, nblocks)))
              for cc in range(0, nblocks, chunk_blocks)]
    sched = dict(
        n_nodes=n_nodes, n_cores=n_cores, S_pc=S_pc, nblocks=nblocks,
        rka=rka, rkb=rkb, RA=RA, ablk=ablk,
        T_lo=T_lo, T_hi=T_hi, cum_lo=cum_lo, cum_hi=cum_hi,
        n_tiles_lo=NT_lo, n_tiles_hi=NT_hi, chunks=chunks,
        dinv=dinv.astype(np.float32), amap=amap, bmap=bmap, bmask=bmask,
    )
    return sched, per_core


def make_inputs(sched, per_core, x, Ws, bs):
    n_cores = sched["n_cores"]
    dinv = sched["dinv"]
    t0 = (np.asarray(x, np.float32) * dinv[:, None]).astype(np.float16)
    xa = np.ascontiguousarray(t0[sched["amap"]])
    xb = t0[sched["bmap"]].copy()
    xb[~sched["bmask"]] = 0
    iota = np.tile(np.arange(128, dtype=np.float16)[None, :], (128, 1))
    in_maps = []
    for r in range(n_cores):
        m = dict(
            xa=xa, xb=xb, iota=iota,
            idx_lo=per_core[r]["idx_lo"], idx_hi=per_core[r]["idx_hi"],
            dloc_lo=per_core[r]["dloc_lo"], dloc_hi=per_core[r]["dloc_hi"],
            dinv_blk=per_core[r]["dinv_blk"],
            dinv2_blk=per_core[r]["dinv2_blk"],
            rdeg=per_core[r]["rdeg"],
        )
        for i, (W, b) in enumerate(zip(Ws, bs)):
            m[f"W{i}"] = np.asarray(W).astype(np.float16)
            m[f"b{i}"] = np.asarray(b, dtype=np.float16)[None, :]
        in_maps.append(m)
    return in_maps


# ---------------------------------------------------------------- device side

def build_nc(sched, fos=(128, 128, 64), n_cores=None, model=False,
             compile=True, reps=1, swdge_queues=4, has_bias=False,
             gmax=16, probe=None):
    import concourse.bacc as bacc
    import concourse.tile as tile
    import concourse.mybir as mybir

    f16, f32, i16 = mybir.dt.float16, mybir.dt.float32, mybir.dt.int16
    N, S_pc = sched["n_nodes"], sched["S_pc"]
    nblocks, rka, rkb = sched["nblocks"], sched["rka"], sched["rkb"]
    ablk = sched["ablk"]
    T_lo, T_hi = sched["T_lo"], sched["T_hi"]
    cum_lo, cum_hi = sched["cum_lo"], sched["cum_hi"]
    NT_lo, NT_hi = sched["n_tiles_lo"], sched["n_tiles_hi"]
    chunks = sched["chunks"]
    n_cores = n_cores or sched["n_cores"]
    n_layers = len(fos)
    NRA, NRB = n_cores * rka * 128, n_cores * rkb * 128   # table rows

    nc = bacc.Bacc("TRN2", target_bir_lowering=False, debug=False,
                   num_devices=n_cores, num_swdge_queues=swdge_queues)

    xa_in = nc.dram_tensor("xa", [NRA, FEAT], f16, kind="ExternalInput")
    xb_in = nc.dram_tensor("xb", [NRB, FEAT], f16, kind="ExternalInput")
    iota_in = nc.dram_tensor("iota", [128, 128], f16, kind="ExternalInput")
    idx_lo_in = nc.dram_tensor("idx_lo", [128, max(NT_lo, 1) * 8], i16, kind="ExternalInput")
    idx_hi_in = nc.dram_tensor("idx_hi", [128, max(NT_hi, 1) * 8], i16, kind="ExternalInput")
    dloc_lo_in = nc.dram_tensor("dloc_lo", [128, max(NT_lo, 1)], f16, kind="ExternalInput")
    dloc_hi_in = nc.dram_tensor("dloc_hi", [128, max(NT_hi, 1)], f16, kind="ExternalInput")
    dinv_in = nc.dram_tensor("dinv_blk", [128, nblocks], f32, kind="ExternalInput")
    dinv2_in = nc.dram_tensor("dinv2_blk", [128, nblocks], f32, kind="ExternalInput")
    if has_bias:
        rdeg_in = nc.dram_tensor("rdeg", [128, nblocks * 128], f16, kind="ExternalInput")
    W_in = [nc.dram_tensor(f"W{i}", [FEAT, fos[i]], f16, kind="ExternalInput")
            for i in range(n_layers)]
    b_in = [nc.dram_tensor(f"b{i}", [1, fos[i]], f16, kind="ExternalInput")
            for i in range(n_layers)]
    y_out = nc.dram_tensor("y", [S_pc, fos[-1]], f32, kind="ExternalOutput")

    rg = [list(range(n_cores))]

    with tile.TileContext(nc) as tc:
        with (
            tc.tile_pool(name="const", bufs=1) as cpool,
            tc.tile_pool(name="sb", bufs=3) as sb,
            tc.tile_pool(name="ps", bufs=6) as pspool,
            tc.tile_pool(name="ab", bufs=4) as abpool,
            tc.tile_pool(name="mbuf", bufs=3) as mbuf,
            tc.tile_pool(name="spool", bufs=3) as spool,
            tc.tile_pool(name="psum_p", bufs=3, space="PSUM") as psum_p,
            tc.tile_pool(name="psum_ab", bufs=3, space="PSUM") as psum_ab,
            tc.tile_pool(name="psum_agg", bufs=2, space="PSUM") as psum_agg,
            tc.tile_pool(name="dram", bufs=2, space="DRAM") as dram,
            tc.tile_pool(name="dram_sh", bufs=2, space="DRAM") as dram_sh,
        ):
            # --- constants
            iota = cpool.tile([128, 128], f16)
            nc.sync.dma_start(out=iota[:], in_=iota_in[:])
            idx_lo = cpool.tile([128, max(NT_lo, 1) * 8], i16)
            nc.sync.dma_start(out=idx_lo[:], in_=idx_lo_in[:])
            idx_hi = cpool.tile([128, max(NT_hi, 1) * 8], i16)
            nc.sync.dma_start(out=idx_hi[:], in_=idx_hi_in[:])
            dloc_lo = cpool.tile([128, max(NT_lo, 1)], f16)
            nc.sync.dma_start(out=dloc_lo[:], in_=dloc_lo_in[:])
            dloc_hi = cpool.tile([128, max(NT_hi, 1)], f16)
            nc.sync.dma_start(out=dloc_hi[:], in_=dloc_hi_in[:])
            dinv_blk = cpool.tile([128, nblocks], f32)
            nc.sync.dma_start(out=dinv_blk[:], in_=dinv_in[:])
            dinv2_blk = cpool.tile([128, nblocks], f32)
            nc.sync.dma_start(out=dinv2_blk[:], in_=dinv2_in[:])
            if has_bias:
                rdeg = cpool.tile([128, nblocks * 128], f16)
                nc.sync.dma_start(out=rdeg[:], in_=rdeg_in[:])
            Wt, bt = [], []
            for i in range(n_layers):
                w = cpool.tile([FEAT, fos[i]], f16, tag=f"W{i}")
                nc.sync.dma_start(out=w[:], in_=W_in[i][:])
                Wt.append(w)
                if has_bias:
                    b = cpool.tile([128, fos[i]], f16, tag=f"b{i}")
                    nc.vector.memset(b[:], 0.0)
                    nc.sync.dma_start(out=b[0:1, :], in_=b_in[i][:])
                    nc.sync.dma_start(out=b[32:33, :], in_=b_in[i][:])
                    bt.append(b)

            # --- two alternating SBUF buffers for region A
            tblA = []
            for i in range(2):
                tA = cpool.tile([128, n_cores * rka * FEAT], f16,
                                tag=f"tblA{i}", name=f"tblA{i}")
                tblA.append(tA)

            def load_tbl(dst, src, rk):
                # src flat (c,p,r) rows of 256B -> dst[p, (c r)*256B]
                nc.sync.dma_start(
                    out=dst[:].rearrange("p (c r f) -> p c r f",
                                         c=n_cores, r=rk),
                    in_=src[:].rearrange("(c p r) f -> p c r f",
                                         p=128, r=rk))

            bq = [0]

            def emit_gathers_a(M, tbl, t0, nt):
                if probe == "nogather":
                    nc.vector.memset(M[:], 0.0)
                    return
                npieces = -(-nt // gmax)
                step = -(-nt // npieces)
                for s0 in range(0, nt, step):
                    sn = min(step, nt - s0)
                    nc.gpsimd.dma_gather(
                        out_ap=M[:, s0 * 128:(s0 + sn) * 128].rearrange(
                            "p (o e) -> p o e", o=1),
                        in_ap=tbl[:],
                        idxs_ap=idx_lo[:, (t0 + s0) * 8:(t0 + s0 + sn) * 8],
                        num_idxs=128 * sn, num_idxs_reg=128 * sn,
                        elem_size=FEAT, transpose=True,
                        sbuf_tokens_per_rank=128,
                        sbuf_free_dim_per_rank=FEAT * 2,
                        single_packet=False, queue_num=0)

            def emit_gathers_b(M, tbl, t0, nt):
                if probe == "nogather":
                    nc.vector.memset(M[:], 0.0)
                    return
                npieces = -(-nt // gmax)
                step = -(-nt // npieces)
                for s0 in range(0, nt, step):
                    sn = min(step, nt - s0)
                    bq[0] = bq[0] % (swdge_queues - 1) + 1
                    nc.gpsimd.dma_gather(
                        out_ap=M[:, s0:s0 + sn, :],
                        in_ap=tbl[:],
                        idxs_ap=idx_hi[:, (t0 + s0) * 8:(t0 + s0 + sn) * 8],
                        num_idxs=128 * sn, num_idxs_reg=128 * sn,
                        elem_size=FEAT,
                        single_packet=False, queue_num=bq[0])

            def emit_S(dloc_t, t0, nt, tag):
                S = spool.tile([128, nt * 128], f16, tag=tag, name=tag)
                in0 = dloc_t[:, t0:t0 + nt].unsqueeze(2).broadcast_to(
                    (128, nt, 128))
                in1 = iota[:].unsqueeze(1).broadcast_to((128, nt, 128))
                nc.vector.tensor_tensor(
                    S[:].rearrange("p (t d) -> p t d", t=nt), in0, in1,
                    mybir.AluOpType.is_equal)
                return S

            GRP = 4
            for rep in range(reps):
                ag_a = ag_b = None
                for l in range(n_layers):
                    fo = fos[l]
                    A = tblA[(rep * n_layers + l) % 2]
                    if l == 0:
                        load_tbl(A, xa_in, rka)
                        # bounce x region B into Internal DRAM for gathers
                        tbB = dram.tile([NRB, FEAT], f16, tag="tbB")
                        nc.sync.dma_start(out=tbB[:], in_=xb_in[:])
                    else:
                        if model:
                            ta = dram.tile([NRA, FEAT], f16, tag="ta")
                            tbB = dram.tile([NRB, FEAT], f16, tag="tbB")
                            nc.sync.dma_start(out=ta[0:rka * 128, :], in_=ag_a[:])
                            nc.sync.dma_start(out=tbB[0:rkb * 128, :], in_=ag_b[:])
                        else:
                            ta = dram_sh.tile([NRA, FEAT], f16, tag="ta",
                                              addr_space="Shared")
                            tbB = dram_sh.tile([NRB, FEAT], f16, tag="tbB",
                                               addr_space="Shared")
                            nc.gpsimd.collective_compute(
                                "AllGather", mybir.AluOpType.bypass,
                                replica_groups=rg,
                                ins=[ag_a[:].opt()], outs=[ta[:].opt()])
                            nc.gpsimd.collective_compute(
                                "AllGather", mybir.AluOpType.bypass,
                                replica_groups=rg,
                                ins=[ag_b[:].opt()], outs=[tbB[:].opt()])
                        load_tbl(A, ta, rka)

                    if l + 1 < n_layers:
                        ag_a = dram.tile([rka * 128, FEAT], f16, tag="ag_a")
                        ag_b = dram.tile([rkb * 128, FEAT], f16, tag="ag_b")
                        ag_a_v = ag_a[:].rearrange("(p r) f -> p r f", p=128)

                    for chunk in chunks:
                        c0 = chunk[0]
                        ctlo = int(sum(T_lo[b] for b in chunk))
                        cthi = int(sum(T_hi[b] for b in chunk))
                        M_lo = M_hi = S_lo = S_hi = None
                        if ctlo:
                            M_lo = mbuf.tile([128, ctlo * 128], f16, tag="Mlo")
                            emit_gathers_a(M_lo, A, int(cum_lo[c0]), ctlo)
                            S_lo = emit_S(dloc_lo, int(cum_lo[c0]), ctlo, "Slo")
                        if cthi:
                            M_hi = mbuf.tile([128, cthi, FEAT], f16, tag="Mhi")
                            emit_gathers_b(M_hi, tbB, int(cum_hi[c0]), cthi)
                            S_hi = emit_S(dloc_hi, int(cum_hi[c0]), cthi, "Shi")

                        # B path: aggB[f, d] per block, then one transform mm
                        aggBs_list = []
                        for b in chunk:
                            nhb = int(T_hi[b])
                            if nhb == 0:
                                aggBs_list.append(None)
                                continue
                            aggB = psum_ab.tile([128, 128], f32)
                            for k in range(nhb):
                                sl = int(cum_hi[b] - cum_hi[c0]) + k
                                nc.tensor.matmul(
                                    aggB[:], M_hi[:, sl, :],
                                    S_hi[:, sl * 128:(sl + 1) * 128],
                                    start=(k == 0), stop=(k == nhb - 1),
                                    skip_group_check=True)
                            aggBs = abpool.tile([128, 128], f16, tag="aggBs")
                            nc.scalar.copy(aggBs[:], aggB[:])
                            aggBs_list.append(aggBs)

                        # A path pass 1: fused transpose+transform per tile,
                        # 4 tiles per PSUM bank, one batched copy per group
                        tinfo = []
                        flat = []
                        for b in chunk:
                            for k in range(int(T_lo[b])):
                                flat.append(int(cum_lo[b] - cum_lo[c0]) + k)
                        for g0 in range(0, len(flat), GRP):
                            grp = flat[g0:g0 + GRP]
                            P = psum_p.tile([128, GRP * fo], f32)
                            for q, sl in enumerate(grp):
                                nc.tensor.matmul(
                                    P[:, q * fo:(q + 1) * fo],
                                    M_lo[:, sl * 128:(sl + 1) * 128],
                                    Wt[l][:], start=True, stop=True,
                                    skip_group_check=True)
                            Ps = pspool.tile([128, GRP * fo], f16, tag="Ps")
                            w = len(grp) * fo
                            if (g0 // GRP) % 2 == 0:
                                nc.scalar.copy(Ps[:, 0:w], P[:, 0:w])
                            else:
                                nc.vector.tensor_scalar_mul(
                                    Ps[:, 0:w], P[:, 0:w], 1.0)
                            for q, sl in enumerate(grp):
                                tinfo.append((sl, Ps, q))

                        # pass 2: per-block accumulation + epilogue
                        ti = 0
                        for bi, b in enumerate(chunk):
                            nlb = int(T_lo[b])
                            agg = psum_agg.tile([128, fo], f32)
                            row = 0 if l + 1 < n_layers else 32
                            first = True
                            if has_bias:
                                nc.tensor.matmul(
                                    agg[:],
                                    rdeg[row:row + 1, b * 128:(b + 1) * 128],
                                    bt[l][row:row + 1, :],
                                    start=True, stop=False,
                                    skip_group_check=True)
                                first = False
                            aggBs = aggBs_list[bi]
                            for j in range(nlb):
                                sl, Ps, q = tinfo[ti]
                                nc.tensor.matmul(
                                    agg[:], S_lo[:, sl * 128:(sl + 1) * 128],
                                    Ps[:, q * fo:(q + 1) * fo],
                                    start=first,
                                    stop=(aggBs is None and j == nlb - 1),
                                    skip_group_check=True)
                                first = False
                                ti += 1
                            if aggBs is not None:
                                nc.tensor.matmul(
                                    agg[:], aggBs[:], Wt[l][:],
                                    start=first, stop=True,
                                    skip_group_check=True)
                            r0 = b * 128
                            rows = min(128, S_pc - r0)
                            if l + 1 < n_layers:
                                hb = sb.tile([128, fo], f16, tag="hb")
                                nc.scalar.activation(
                                    hb[:], agg[:],
                                    mybir.ActivationFunctionType.Relu,
                                    scale=dinv2_blk[:, b:b + 1])
                                if b < ablk:
                                    nc.sync.dma_start(
                                        out=ag_a_v[:, b, :], in_=hb[:])
                                else:
                                    rb0 = (b - ablk) * 128
                                    nc.sync.dma_start(
                                        out=ag_b[rb0:rb0 + rows, :],
                                        in_=hb[0:rows, :])
                            else:
                                yb = sb.tile([128, fo], f32, tag="yb")
                                nc.scalar.activation(
                                    yb[:], agg[:],
                                    mybir.ActivationFunctionType.Copy,
                                    scale=dinv_blk[:, b:b + 1])
                                nc.sync.dma_start(
                                    out=y_out[r0:r0 + rows, :],
                                    in_=yb[0:rows, :])
    if compile:
        nc.compile()
    return nc


# ---------------------------------------------------------------- entry point

_CACHE = {}


def kernel(x, edge_index, W1, b1, W2, b2, W3, b3):
    import sys
    if "/opt/trn_rl_repo" not in sys.path:
        sys.path.insert(0, "/opt/trn_rl_repo")
    from concourse import bass_utils

    x = np.asarray(x)
    edge_index = np.asarray(edge_index)
    Ws = [np.asarray(W1), np.asarray(W2), np.asarray(W3)]
    bs = [np.asarray(b1), np.asarray(b2), np.asarray(b3)]
    n = x.shape[0]
    has_bias = any(np.any(np.asarray(b) != 0) for b in bs)

    key = (n, edge_index.shape[1], has_bias)
    if key in _CACHE and np.array_equal(_CACHE[key][0], edge_index):
        _, sched, per_core, nc = _CACHE[key]
    else:
        sched, per_core = preprocess(edge_index, n, N_CORES)
        nc = build_nc(sched, fos=(W1.shape[1], W2.shape[1], W3.shape[1]),
                      has_bias=has_bias)
        _CACHE[key] = (edge_index.copy(), sched, per_core, nc)

    in_maps = make_inputs(sched, per_core, x, Ws, bs)
    if not has_bias:
        for m in in_maps:
            m.pop("rdeg", None)
    res = bass_utils.run_bass_kernel_spmd(nc, in_maps,
                                          core_ids=list(range(N_CORES)))
    out = np.concatenate([res.results[r]["y"] for r in range(N_CORES)], axis=0)
    return out.astype(np.float32)
